# revision 92
# baseline (speedup 1.0000x reference)
"""Trainium2 Bass kernel for nn_Matcher (retrieval_knn), v2.

Computation (per batch b):
  c1 = concat([src1, nn(src1->tar1)])        # [2048, 64, 64]
  c2 = concat([src2, nn(src2->tar2)])        # [4096, 32, 32]
  out = concat([c1, bilinear_up2x(c2)])      # [6144, 64, 64]
where nn(s->t)[p] = t[:, argmin_j ||s[:,p]-t[:,j]||^2].

Sharding: 8 cores = 4 batches x 2 source-pixel halves.  Each core owns a
contiguous half of the level-1 source pixels (2048 of 4096) and an
18-row window of the level-2 source grid, so the argmin is fully local
(no collectives) and the core emits the bilinear-upsampled output rows
32h..32h+31 by itself.

Design (vs the v1 two/three-pass kernel; 1.88ms -> ~0.53ms):
- Host ships layout-transformed inputs only (casts/transposes/slices):
  bf16 channel-chunked t and s for the GEMM, fp32 pixel-major s for the
  rescore, fp32 row-major t for the gathers, and the (constant)
  bilinear-interpolation weight tiles.
- Both levels run a single bf16 GEMM of v = s.t - |t|^2/2 with the
  -|t|^2/2 term folded in as one extra K=2 matmul (bf16 hi/lo pair of
  the device-computed row norms; norms from bf16 squares, validated to
  keep the true winner within the top-2 with >=0.037 margin).
- Top-2 candidates are rescored exactly in fp32: two indirect-DMA row
  gathers, a fused dot (scalar_tensor_tensor with accum), |g|^2 via
  ACT Square-accum (L1) / DVE (L2), then a per-pixel mask select.
- The bilinear 2x upsample is a sparse-weight matmul on the Tensor
  engine (out-pixel blocks x channel blocks, contraction over the 576
  window pixels), consuming the pixel-major s2/near2 tiles directly.
- Outputs leave the device as bf16 pixel-major (1.8e-3 output rel err,
  vs the 2e-2 gate); the host widens/transposes into the fp32 result.

Scheduling notes (what the ~3x over v1 came from):
- Keep the PE dense so the HAM clock gate stays at 2.4 GHz: no
  on-chip transposes (layouts shipped), staging prefetched two m-tiles
  ahead across the level boundary, upsample s2-halves front-run each
  level-2 GEMM as dependency-free filler, near-halves trail two
  m-tiles behind their rescore.
- |t|^2 rows ride in partition 0 of the shared psum rotation (r1 in
  four quarters up-front on ACT+DVE; r2 spread over level-1 m-tiles
  4..11 on the idle gpsimd engine, accumulated in SBUF).
- Per-engine FIFO head-of-line blocking is the main scheduling hazard:
  near1 output DMAs go out on the gpsimd queue, psum is evacuated in
  2-bank pairs by ACT, scratch tiles rotate per-tag, and the rescore
  work is split across DVE/ACT/gpsimd to keep each queue's waits short.
"""

import sys

sys.path.insert(0, "/opt/trn_rl_repo")

import copy
import numpy as np
import ml_dtypes

import concourse.bass as bass
import concourse.mybir as mybir
import concourse.tile as tile
import concourse.tile_utils as tile_utils
from concourse.vector_clock import ScopedClock

F32 = mybir.dt.float32
BF16 = mybir.dt.bfloat16
U32 = mybir.dt.uint32
SQUARE = mybir.ActivationFunctionType.Square
COPYF = mybir.ActivationFunctionType.Copy
MULT = mybir.AluOpType.mult
ADD = mybir.AluOpType.add
IS_GT = mybir.AluOpType.is_gt

NPBF16 = ml_dtypes.bfloat16

# ---------------------------------------------------------------------------
# Toolchain workarounds for this walrus build.
# ---------------------------------------------------------------------------

# cayman SBUF: 224 KiB active minus the 16 KiB SWDGE descriptor carveout
tile_utils.max_sbuf_usage = int(207.5 * 1024)


def _patched_drain_and_barrier(self, tick_clock, wait_clock):
    nc = self.nc
    drain_inst = nc.sync.drain()
    wait_clock.add_sem_waits(
        drain_inst.ins, ScopedClock({None: tick_clock.global_clock})
    )
    nc.all_engine_barrier()
    assert self.sems is not None
    popped = nc._tile_sem_poison_stack.pop()
    assert popped is self._sem_poison
    nc.clear_and_free_semaphores(list(self.sems.allocated().values()))
    nc.all_engine_barrier()


tile.TileContext._drain_and_barrier = _patched_drain_and_barrier


def split_sync_waits(nc, maxw=1):
    """walrus rejects instructions carrying more than a couple of sync
    waits; hoist the excess onto nofuse nops inserted just before."""
    tmpl = nc.sync.nop(nofuse=True)
    tmpl_name = tmpl.ins.name
    template = copy.deepcopy(tmpl.ins)
    counter = [0]

    def make_nop(engine, waits):
        n = copy.deepcopy(template)
        counter[0] += 1
        n.name = f"I-wsplit-{counter[0]}"
        n.engine = engine
        n.sync_info = mybir.SyncInfo(on_wait=list(waits), on_update=[])
        return n

    for f in nc.m.functions:
        for bb in f.blocks:
            out = []
            changed = False
            for ins in bb.instructions:
                if ins.name == tmpl_name:
                    changed = True
                    continue
                si = ins.sync_info
                if si is not None and len(si.on_wait) > maxw:
                    waits = list(si.on_wait)
                    for i in range(0, len(waits) - maxw, maxw):
                        out.append(make_nop(ins.engine, waits[i : i + maxw]))
                    si.on_wait = waits[len(waits) - maxw :]
                    changed = True
                out.append(ins)
            if changed:
                bb.instructions = out


# ---------------------------------------------------------------------------
# Bilinear-upsample weight tiling (h-independent metadata, per-h weights)
# ---------------------------------------------------------------------------


def _ups_scheme():
    """Per out-pixel block i (2 out rows x 64 cols = 128 opix), the fixed
    list of (tile_idx, window_chunk, K) sub-matmuls.  Every sub-matmul
    contracts over the chunk's full partition range from partition 0
    (matmul cost is independent of K; unused rows carry zero weights)."""
    scheme = []
    t = 0
    for i in range(16):
        c0, r = divmod(i, 4)
        chunks = [c0] if r <= 1 else [c0, c0 + 1]
        out = []
        for ch in chunks:
            out.append((t, ch, 64 if ch == 4 else 128))
            t += 1
        scheme.append(out)
    return scheme, t


_UPS_SCHEME, _UPS_T = _ups_scheme()
# blocks whose near-half becomes computable after level-2 m-tile m completes
# (max window chunk == m)
_UPS_BLOCKS_AFTER = [[0, 1], [2, 3, 4, 5], [6, 7, 8, 9], [10, 11, 12, 13], [14, 15]]



def _ups_weights(h):
    """wup [128, T, 128] fp32 weight tiles for core half h."""
    Wv = np.zeros((32, 18), np.float64)
    for R in range(32):
        p = min(max((32 * h + R + 0.5) / 2 - 0.5, 0.0), 31.0)
        r0 = int(np.floor(p))
        r1 = min(r0 + 1, 31)
        f = p - r0
        Wv[R, r0 - 16 * h + 1] += 1.0 - f
        Wv[R, r1 - 16 * h + 1] += f
    Wh = np.zeros((64, 32), np.float64)
    for C in range(64):
        q = min(max((C + 0.5) / 2 - 0.5, 0.0), 31.0)
        c0 = int(np.floor(q))
        c1 = min(c0 + 1, 31)
        f = q - c0
        Wh[C, c0] += 1.0 - f
        Wh[C, c1] += f
    wup = np.zeros((128, _UPS_T, 128), np.float64)
    for i, subs in enumerate(_UPS_SCHEME):
        for t, ch, K in subs:
            for wloc in range(K // 32):
                w = 4 * ch + wloc
                if w >= 18:
                    continue
                rows = slice(32 * wloc, 32 * wloc + 32)
                for Rl in range(2):
                    wv = Wv[2 * i + Rl, w]
                    if wv == 0.0:
                        continue
                    # [32 in-cols, 64 out-cols]
                    wup[rows, t, Rl * 64 : (Rl + 1) * 64] = wv * Wh.T
    return np.ascontiguousarray(wup.astype(NPBF16))


# ---------------------------------------------------------------------------
# Device program
# ---------------------------------------------------------------------------


def _emit_knn_mtile(nc, tc, pools, K, N, C, th, rhl, ones2, tr_d, near_out,
                    v_ap=None):
    """One KNN m-tile: GEMM + top-2 + exact rescore + select.
    th: [128, K, N] bf16 SBUF.
    near_out: bf16 [128, C] AP to fill, or None to allocate (returned).
    sh/sp are pre-staged SBUF tiles (DMA'd ahead by the caller).
    vpool is a shared max-shape pool; its tiles are sliced here."""
    psum, vpool, gpool, small, scrp, nearp, sh, sp, rr_eng = pools
    NT = N // 512
    BYP = mybir.AluOpType.bypass

    # level 2 packs its [128, 1024] v into quarter-slots of one held
    # [128, 4096] tile (v_ap), so its GEMM/evac never wait on older chains
    if v_ap is not None:
        v = v_ap
    else:
        v_t = vpool.tile([128, 4096], F32, tag="v")
        v = v_t[:, :N]
    for nbp in range(NT // 2):
        pv = psum.tile([128, 1024], F32, tag="mm2")
        for sub in range(2):
            nb = 2 * nbp + sub
            ns = slice(nb * 512, (nb + 1) * 512)
            pvs = pv[:, sub * 512 : (sub + 1) * 512]
            for k in range(K):
                nc.tensor.matmul(pvs, sh[:, k], th[:, k, ns], start=(k == 0),
                                 stop=False)
            nc.tensor.matmul(pvs, ones2, rhl[:, ns], start=False, stop=True)
        # one evacuation per 2-bank pair keeps the ACT queue short
        nc.scalar.copy(v[:, nbp * 1024 : (nbp + 1) * 1024], pv)

    m8 = small.tile([128, 8], F32, tag="m8")
    i8 = small.tile([128, 8], U32, tag="i8")
    nc.vector.max(out=m8, in_=v)
    nc.vector.max_index(out=i8, in_max=m8, in_values=v)

    g = []
    for c in range(2):
        gc = gpool.tile([128, C], F32, tag=f"g{c}")
        nc.gpsimd.indirect_dma_start(
            out=gc[:], out_offset=None, in_=tr_d,
            in_offset=bass.IndirectOffsetOnAxis(ap=i8[:, c : c + 1], axis=0),
        )
        g.append(gc)

    dots = small.tile([128, 2], F32, tag="dots")
    rr = small.tile([128, 2], F32, tag="rr")
    score = small.tile([128, 2], F32, tag="score")
    for c in range(2):
        sA = scrp.tile([128, C], F32, tag="sA")
        nc.vector.scalar_tensor_tensor(
            out=sA, in0=g[c], scalar=0.0, in1=sp, op0=BYP, op1=MULT,
            accum_out=dots[:, c : c + 1],
        )
        if rr_eng == "act":
            sB = scrp.tile([128, C], F32, tag="sB")
            nc.scalar.activation(sB, g[c], SQUARE, accum_out=rr[:, c : c + 1])
        else:
            sA2 = scrp.tile([128, C], F32, tag="sA")
            nc.vector.scalar_tensor_tensor(
                out=sA2, in0=g[c], scalar=-1.0, in1=g[c], op0=MULT, op1=MULT,
                accum_out=rr[:, c : c + 1],
            )
    # score = dots - rr/2   (rr holds +|g|^2 on ACT, -|g|^2 on DVE)
    nc.vector.tensor_scalar(out=score, in0=rr,
                            scalar1=(-0.5 if rr_eng == "act" else 0.5),
                            scalar2=None, op0=MULT)
    nc.vector.tensor_add(score, score, dots)
    mask = small.tile([128, 1], F32, tag="mask")
    nc.vector.tensor_tensor(out=mask, in0=score[:, 1:2], in1=score[:, 0:1], op=IS_GT)
    # near = g0 + mask * (g1 - g0), emitted in bf16
    diff = scrp.tile([128, C], F32, tag="sA")
    nc.gpsimd.tensor_sub(diff, g[1], g[0])
    if near_out is None:
        near_out = nearp.tile([128, C], BF16, tag="near")
    nc.vector.scalar_tensor_tensor(
        out=near_out, in0=diff, scalar=mask[:, 0:1], in1=g[0], op0=MULT, op1=ADD,
    )
    return near_out


def _emit_r_chunk(nc, pr, th, k, first, last, qoff, ones_col, apool, engine):
    """One k-chunk of one [1, 1024] quarter of the -|t|^2/2 reduction:
    square (on `engine`) then ones-matmul partition-reduce into `pr`.
    first/last control the psum accumulation group."""
    for nb2 in range(2):
        ns = slice(qoff + nb2 * 512, qoff + (nb2 + 1) * 512)
        sq = apool.tile([128, 512], BF16, tag="sq")
        if engine == "act":
            nc.scalar.activation(sq, th[:, k, ns], SQUARE)
        elif engine == "dve":
            nc.vector.tensor_mul(sq, th[:, k, ns], th[:, k, ns])
        else:
            nc.gpsimd.tensor_mul(sq, th[:, k, ns], th[:, k, ns])
        nc.tensor.matmul(
            pr[:, nb2 * 512 : (nb2 + 1) * 512], ones_col, sq[:],
            start=first, stop=last,
        )


def _emit_r_finish(nc, pr, rhl_q, rpool):
    """Finalize one [1, 1024] quarter: rhl_q = bf16 hi/lo pair of -pr/2."""
    nc.scalar.activation(rhl_q[0:1, :], pr, COPYF, scale=-0.5)
    rl = rpool.tile([1, 1024], BF16, tag="rl")
    # rl = (-pr/2) - rh, then DMA across to partition 1
    nc.vector.scalar_tensor_tensor(
        out=rl, in0=pr, scalar=-0.5, in1=rhl_q[0:1, :],
        op0=MULT, op1=mybir.AluOpType.subtract,
    )
    nc.sync.dma_start(rhl_q[1:2, :], rl[:])


def build_program():
    from contextlib import ExitStack

    nc = bass.Bass()

    th1_d = nc.dram_tensor("th1", [128, 8, 4096], BF16, kind="ExternalInput")
    s1h_d = nc.dram_tensor("s1h", [128, 8, 16, 128], BF16, kind="ExternalInput")
    s1p_d = nc.dram_tensor("s1p", [128, 16, 1024], F32, kind="ExternalInput")
    tr1_d = nc.dram_tensor("tr1", [4096, 1024], F32, kind="ExternalInput")
    th2_d = nc.dram_tensor("th2", [128, 16, 1024], BF16, kind="ExternalInput")
    s2h_d = nc.dram_tensor("s2h", [128, 16, 5, 128], BF16, kind="ExternalInput")
    s2p_d = nc.dram_tensor("s2p", [128, 5, 2048], F32, kind="ExternalInput")
    tr2_d = nc.dram_tensor("tr2", [1024, 2048], F32, kind="ExternalInput")
    wup_d = nc.dram_tensor("wup", [128, _UPS_T, 128], BF16, kind="ExternalInput")

    near1_d = nc.dram_tensor("near1", [2048, 1024], BF16, kind="ExternalOutput")
    up_d = nc.dram_tensor("up", [2048, 4096], BF16, kind="ExternalOutput")

    with tile.TileContext(nc) as tc:
        with ExitStack() as top:
            const = top.enter_context(tc.tile_pool(name="const", bufs=1))
            ones_col = const.tile([128, 1], BF16)
            nc.vector.memset(ones_col, 1.0)
            ones2 = const.tile([2, 128], BF16)
            nc.vector.memset(ones2, 1.0)
            rhl1 = const.tile([2, 4096], BF16)
            rhl2 = const.tile([2, 1024], BF16)

            # Pools shared across both levels (allocated once, at top scope):
            # level-2 staging prefetches and its GEMM/evac start the moment
            # level-1 drains, without waiting for a freed SBUF region.
            th2p = top.enter_context(tc.tile_pool(name="th2p", bufs=1))
            th2 = th2p.tile([128, 16, 1024], BF16)
            wup = th2p.tile([128, _UPS_T, 128], BF16)
            nc.sync.dma_start(wup, wup_d[:])
            acc2 = th2p.tile([1, 1024], F32)
            vpool = top.enter_context(tc.tile_pool(name="vbuf", bufs=2))
            shp = top.enter_context(tc.tile_pool(name="shstage", bufs=3))
            spp = top.enter_context(tc.tile_pool(name="spstage", bufs=2))
            rfin = top.enter_context(tc.tile_pool(name="rfin", bufs=1))
            psum = top.enter_context(tc.tile_pool(name="psum", bufs=3, space="PSUM"))

            # unified m-tile schedule across both levels, with staging
            # prefetched two tiles ahead (crossing the level boundary)
            M_L1, M_L2 = 16, 5
            tiles = [("L1", m) for m in range(M_L1)] + [("L2", m) for m in range(M_L2)]

            s2ph_holder = []
            uncast = []

            def stage(idx):
                lvl, m = tiles[idx]
                sh = shp.tile([128, 16, 128], BF16, tag="sh")
                sp = spp.tile([128, 2048], F32, tag="sp")
                if lvl == "L1":
                    nc.sync.dma_start(sh[:, :8, :], s1h_d[:, :, m, :])
                    nc.sync.dma_start(sp[:, :1024], s1p_d[:, m, :])
                else:
                    nc.sync.dma_start(sh, s2h_d[:, :, m, :])
                    nc.sync.dma_start(sp, s2p_d[:, m, :])
                    if s2ph_holder:
                        # pre-cast the s2 pixel chunk for the upsample so its
                        # s2-halves can front-run the level-2 GEMMs
                        nc.scalar.copy(s2ph_holder[0][:, m, :], sp)
                    else:
                        uncast.append((m, sp))
                return sh, sp

            # ======================= Level 1 =======================
            with ExitStack() as l1:
                l1p = l1.enter_context(tc.tile_pool(name="l1p", bufs=1))
                th1 = l1p.tile([128, 8, 4096], BF16)
                for k in range(8):
                    nc.sync.dma_start(th1[:, k], th1_d[:, k])
                staged = {}
                staged[0] = stage(0)
                staged[1] = stage(1)

                # r1 up-front in four [1,1024] quarters living in partition 0
                # of the shared psum rotation; squares alternate ACT/DVE
                # (both idle at start) so the feed outruns the PE.
                with tc.tile_pool(name="r1a", bufs=4) as apool:
                    for q in range(4):
                        prt = psum.tile([128, 1024], F32, tag="mm2")
                        pr1 = prt[0:1, :]
                        for k in range(8):
                            _emit_r_chunk(nc, pr1, th1, k, k == 0, k == 7,
                                          q * 1024, ones_col, apool,
                                          "act" if k % 2 == 0 else "dve")
                        _emit_r_finish(nc, pr1, rhl1[:, q * 1024 : (q + 1) * 1024],
                                       rfin)

                scrp = l1.enter_context(tc.tile_pool(name="c1scr", bufs=1))
                gpool = l1.enter_context(tc.tile_pool(name="c1g", bufs=2))
                small = l1.enter_context(tc.tile_pool(name="c1small", bufs=2))
                nearp = l1.enter_context(tc.tile_pool(name="c1near", bufs=1))

                # r2 partial sums ride in partition 0 of a transient psum
                # tile per level-1 m-tile (4..11), accumulated into acc2 in
                # SBUF; squares run on the idle gpsimd engine.
                nc.vector.memset(acc2, 0.0)
                r2actx = ExitStack()
                r2a = r2actx.enter_context(tc.tile_pool(name="r2a", bufs=2))

                for m in range(16):
                    sh, sp = staged.pop(m)
                    if m + 2 < len(tiles):
                        staged[m + 2] = stage(m + 2)
                    pools = (psum, vpool, gpool, small, scrp, nearp,
                             sh[:, :8, :], sp[:, :1024], "act")
                    near = _emit_knn_mtile(
                        nc, tc, pools, 8, 4096, 1024, th1, rhl1, ones2,
                        tr1_d[:], None,
                    )
                    nc.gpsimd.dma_start(near1_d[m * 128 : (m + 1) * 128, :], near)
                    if m == 1:
                        # th2 loads ride behind the level-1 staging traffic;
                        # first needed by the r2 chunks at m == 4.
                        for k in range(16):
                            nc.sync.dma_start(th2[:, k], th2_d[:, k])
                    if 4 <= m < 12:
                        prt = psum.tile([128, 1024], F32, tag="mm2")
                        pr2 = prt[0:1, :]
                        for j, k in enumerate((2 * (m - 4), 2 * (m - 4) + 1)):
                            _emit_r_chunk(nc, pr2, th2, k, j == 0, j == 1, 0,
                                          ones_col, r2a, "gpsimd")
                        nc.vector.tensor_add(acc2, acc2, pr2)
                    elif m == 12:
                        r2actx.close()
                        _emit_r_finish(nc, acc2, rhl2, rfin)

            # ======================= Level 2 + upsample =======================
            with ExitStack() as l2:
                l2p = l2.enter_context(tc.tile_pool(name="l2p", bufs=1))
                near2ph = l2p.tile([128, 5, 2048], BF16)
                s2ph = l2p.tile([128, 5, 2048], BF16)
                s2ph_holder.append(s2ph)
                for m, sp_u in uncast:
                    nc.scalar.copy(s2ph[:, m, :], sp_u)
                uncast.clear()

                scrp = l2.enter_context(tc.tile_pool(name="c2scr", bufs=1))
                gpool = l2.enter_context(tc.tile_pool(name="c2g", bufs=2))
                small = l2.enter_context(tc.tile_pool(name="c2small", bufs=4))
                upool = l2.enter_context(tc.tile_pool(name="ups", bufs=2))
                upsum = l2.enter_context(tc.tile_pool(name="upsum", bufs=2,
                                                      space="PSUM"))

                def emit_ups_half(blk, half):
                    # half 0: src2 channels (PE filler once its chunk is
                    # cast); half 1: nearest channels (needs near2ph chunks)
                    src = s2ph if half == 0 else near2ph
                    ut = upool.tile([128, 2048], BF16, tag="upsb")
                    subs = _UPS_SCHEME[blk]
                    for nb in range(4):
                        cho = nb * 512
                        pu = upsum.tile([128, 512], F32, tag="up")
                        for si, (t, ch, K) in enumerate(subs):
                            nc.tensor.matmul(
                                pu,
                                wup[0:K, t, :],
                                src[0:K, ch, cho : cho + 512],
                                start=(si == 0), stop=(si == len(subs) - 1),
                            )
                        nc.scalar.copy(ut[:, cho : cho + 512], pu)
                    nc.sync.dma_start(
                        up_d[blk * 128 : (blk + 1) * 128,
                             half * 2048 : (half + 1) * 2048],
                        ut,
                    )

                # one held v tile provides five independent quarter-slots
                # (slot 4 reuses slot 0 after its scans complete)
                v2all = vpool.tile([128, 4096], F32, tag="v")
                for m in range(5):
                    # s2-halves of already-cast chunks front-run the GEMM as
                    # dependency-free PE filler, keeping the PE warm through
                    # the level-2 rescore chains.
                    for blk in _UPS_BLOCKS_AFTER[m]:
                        emit_ups_half(blk, 0)
                    sh, sp = staged.pop(16 + m)
                    if 16 + m + 2 < len(tiles):
                        staged[16 + m + 2] = stage(16 + m + 2)
                    pools = (psum, vpool, gpool, small, scrp, None, sh, sp,
                             "dve")
                    off = (m % 4) * 1024
                    _emit_knn_mtile(
                        nc, tc, pools, 16, 1024, 2048, th2, rhl2, ones2,
                        tr2_d[:], near2ph[:, m, :],
                        v_ap=v2all[:, off : off + 1024],
                    )
                    if m >= 2:
                        for blk in _UPS_BLOCKS_AFTER[m - 2]:
                            emit_ups_half(blk, 1)
                for m in (3, 4):
                    for blk in _UPS_BLOCKS_AFTER[m]:
                        emit_ups_half(blk, 1)

    split_sync_waits(nc)
    return nc


_NC_CACHE = None


def _get_nc():
    global _NC_CACHE
    if _NC_CACHE is None:
        _NC_CACHE = build_program()
    return _NC_CACHE


# ---------------------------------------------------------------------------
# Host-side sharding / layout prep
# ---------------------------------------------------------------------------


def _shard_inputs(src_feat1, tar_feat1, src_feat2, tar_feat2):
    per_batch = []
    for b in range(4):
        t1 = tar_feat1[b].reshape(1024, 4096)
        th1 = np.ascontiguousarray(
            t1.astype(NPBF16).reshape(8, 128, 4096).transpose(1, 0, 2)
        )
        tr1 = np.ascontiguousarray(t1.T)
        t2 = tar_feat2[b].reshape(2048, 1024)
        th2 = np.ascontiguousarray(
            t2.astype(NPBF16).reshape(16, 128, 1024).transpose(1, 0, 2)
        )
        tr2 = np.ascontiguousarray(t2.T)
        per_batch.append((th1, tr1, th2, tr2))

    wups = [_ups_weights(0), _ups_weights(1)]

    in_maps = []
    for core in range(8):
        b, h = core // 2, core % 2
        th1, tr1, th2, tr2 = per_batch[b]
        s1 = src_feat1[b].reshape(1024, 4096)[:, h * 2048 : (h + 1) * 2048]
        s1h = np.ascontiguousarray(
            s1.astype(NPBF16).reshape(8, 128, 16, 128).transpose(1, 0, 2, 3)
        )
        s1p = np.ascontiguousarray(
            s1.T.reshape(16, 128, 1024).transpose(1, 0, 2)
        )
        rows = np.clip(np.arange(16 * h - 1, 16 * h + 17), 0, 31)
        s2w = src_feat2[b].reshape(2048, 32, 32)[:, rows, :].reshape(2048, 576)
        s2wp = np.zeros((2048, 640), np.float32)
        s2wp[:, :576] = s2w
        s2h = np.ascontiguousarray(
            s2wp.astype(NPBF16).reshape(16, 128, 5, 128).transpose(1, 0, 2, 3)
        )
        s2p = np.ascontiguousarray(
            s2wp.T.reshape(5, 128, 2048).transpose(1, 0, 2)
        )
        s2ph = np.ascontiguousarray(s2p.astype(NPBF16))
        in_maps.append({
            "th1": th1, "s1h": s1h, "s1p": s1p, "tr1": tr1,
            "th2": th2, "s2h": s2h, "s2p": s2p, "s2ph": s2ph, "tr2": tr2,
            "wup": wups[h],
        })
    return in_maps


def kernel(src_feat1, tar_feat1, src_feat2, tar_feat2):
    from concourse.bass_utils import run_bass_kernel_spmd

    src_feat1 = np.ascontiguousarray(src_feat1, dtype=np.float32)
    tar_feat1 = np.ascontiguousarray(tar_feat1, dtype=np.float32)
    src_feat2 = np.ascontiguousarray(src_feat2, dtype=np.float32)
    tar_feat2 = np.ascontiguousarray(tar_feat2, dtype=np.float32)

    nc = _get_nc()
    in_maps = _shard_inputs(src_feat1, tar_feat1, src_feat2, tar_feat2)
    res = run_bass_kernel_spmd(nc, in_maps, core_ids=list(range(8)))

    out = np.empty((4, 6144, 64, 64), np.float32)
    for core in range(8):
        b, h = core // 2, core % 2
        r = res.results[core]
        out[b, 0:1024] = src_feat1[b]
        near1 = np.asarray(r["near1"]).astype(np.float32)  # [2048 pix, 1024 ch]
        out[b, 1024:2048].reshape(1024, 4096)[:, h * 2048 : (h + 1) * 2048] = near1.T
        up = np.asarray(r["up"]).astype(np.float32)        # [2048 opix, 4096 ch]
        out[b, 2048:6144, 32 * h : 32 * (h + 1), :] = up.T.reshape(4096, 32, 64)
    return out
